# revision 2
# baseline (speedup 1.0000x reference)
"""Trainium2 Bass kernel v2 for nn_AttentionLayer (B=4, S=2048, H=16, DH=64).

Sharding: 8 cores = 4 batches x 2 head-halves. Core c: batch c//2, heads
(c%2)*8..+8 (512 of 1024 QKV columns).

Per-core design (engine-balanced; rel_rms ~5e-3):
  - QKV projections: fp8e4 DoubleRow matmuls (2 contraction planes/pass =
    4x f32r rate); x and W each split hi+lo fp8, keeping 3 of 4 terms
    (xh@Wh + xl@Wh + xh@Wl) -> ~0.2% proj error at 61us PE (vs 82 bf16).
  - scores: bf16 q/k (fp8 q/k measured 2e-2 -- too lossy), 1 cyc/row.
  - exp: E = 4*exp(s/8) -> bf16 (fp8 E measured 1.8e-2 -- too lossy),
    split between ACT (native Exp) and a custom single-uop DVE op
    (Horner quadratic + 4 squarings: q(s)^16, minimax |rel| <= 1.3%).
  - PV: bf16, q-on-partitions (ctx [128q, 65] per q-tile; v column 64 is
    ones so PV accumulates the softmax denominator for free).
  - normalize: DVE reciprocal [128,8] + 8 tensor_scalar muls per unit.
Output is [S, 512] per core (no host transpose).
"""

import numpy as np
import ml_dtypes

B, S, H, DH = 4, 2048, 16, 64
D = H * DH
NCORES = 8
COLS = 512
NKB = S // 128
QG = 1024
NQG = S // QG
NPAIR = 4
WS = 32.0

EXP_A = 0.005756143013038364
EXP_B = 1.4930625826843371
EXP_D = 1.0906756739585155
LN_ESCALE = float(np.log(4.0))

# k-blocks exp'd on ACT; the rest use the DVE custom op (alternating so the
# two engines run each unit's exp chain in parallel)
ACT_KBS = frozenset({0, 1, 2, 4, 5, 7, 8, 10, 11, 13, 14})  # 11 of 16

F8NP = ml_dtypes.float8_e4m3
BFNP = ml_dtypes.bfloat16

_CACHE = {}


def _register_exp16():
    import concourse.dve_ops as dve_ops
    from concourse.dve_spec import Spec, Src0, C0, C1, C2, sq, lower
    from concourse.dve_uop import DveOpSpec

    name = "EXP16_ATTN"
    for op in dve_ops.OPS:
        if op.name == name:
            return op
    t1 = Src0 * C0
    body = sq(sq(sq(sq((t1 + C1) * t1 + C2))))

    def ref(in0, in1, s0, s1, imm2):
        t = in0.astype(np.float32) * s0
        q = (t + s1) * t + imm2
        return (((q * q) ** 2) ** 2) ** 2

    spec = Spec(body=body, reference=ref)
    row = dve_ops._CUSTOM_DVE_ROW_BASE + len(dve_ops.OPS)
    shas = {}
    for ver in ("v3", "v4"):
        uops = lower(spec, ver=ver)
        shas[ver] = DveOpSpec(name=name, opcode=row, uops=uops, rd1_en=False).sha(
            ver
        )
    op = dve_ops.DveOp(name, spec, subdim=False, uops_sha=shas)
    dve_ops.OPS.append(op)
    dve_ops.CUSTOM_DVE_SPECS[name] = spec
    dve_ops._SUB_OPCODE_FOR_NAME[name] = row
    return op


def _build():
    import concourse.mybir as mybir
    import concourse.tile as tile
    from concourse import bacc

    exp16 = _register_exp16()

    f32 = mybir.dt.float32
    bf16 = mybir.dt.bfloat16
    fp8 = mybir.dt.float8e4
    DR = mybir.MatmulPerfMode.DoubleRow
    Exp = mybir.ActivationFunctionType.Exp
    mult = mybir.AluOpType.mult
    add = mybir.AluOpType.add

    nc = bacc.Bacc(
        "TRN2",
        target_bir_lowering=False,
        debug=False,
        enable_asserts=False,
        num_devices=NCORES,
    )

    xh_d = nc.dram_tensor("xh", [D, S], fp8, kind="ExternalInput").ap()
    xl_d = nc.dram_tensor("xl", [D, S], fp8, kind="ExternalInput").ap()
    wd = {
        n: nc.dram_tensor(n, [D, COLS], fp8, kind="ExternalInput").ap()
        for n in ("wqh", "wql", "wkh", "wkl", "wvh", "wvl")
    }
    bq_d = nc.dram_tensor("bq", [COLS], f32, kind="ExternalInput").ap()
    bk_d = nc.dram_tensor("bk", [COLS], f32, kind="ExternalInput").ap()
    bv_d = nc.dram_tensor("bv", [COLS], f32, kind="ExternalInput").ap()
    out_d = nc.dram_tensor("out", [S, COLS], f32, kind="ExternalOutput").ap()

    with tile.TileContext(nc) as tc:
        with (
            tc.tile_pool(name="consts", bufs=1) as consts,
            tc.tile_pool(name="wpool", bufs=1) as wpool,
            tc.tile_pool(name="xpool", bufs=1) as xpool,
            tc.tile_pool(name="qkt", bufs=1) as qkt,
            tc.tile_pool(name="vpool", bufs=1) as vpool,
            tc.tile_pool(name="epool", bufs=17) as epool,
            tc.tile_pool(name="opool", bufs=4) as opool,
            tc.tile_pool(name="psum", bufs=1, space="PSUM") as psum,
        ):
            # ---- constants ----
            bq_t = consts.tile([128, NPAIR], f32)  # [p, pair]
            bk_t = consts.tile([128, NPAIR], f32)
            nc.gpsimd.dma_start(out=bq_t, in_=bq_d.rearrange("(m p) -> p m", p=128))
            nc.gpsimd.dma_start(out=bk_t, in_=bk_d.rearrange("(m p) -> p m", p=128))
            bv_s = consts.tile([1, COLS], f32)
            nc.gpsimd.dma_start(out=bv_s, in_=bv_d[None, :])
            bvb = consts.tile([128, 2, COLS], f32)
            nc.gpsimd.partition_broadcast(bvb[:, 0, :], bv_s)
            nc.gpsimd.partition_broadcast(bvb[:, 1, :], bv_s)
            ln4 = consts.tile([128, 1], f32)
            nc.vector.memset(ln4, LN_ESCALE)
            warm = consts.tile([1, 1], f32)
            nc.vector.memset(warm, 0.0)
            nc.scalar.activation(warm, warm, Exp)

            # ---- persistent fp8 operands ----
            wt = {
                n: wpool.tile([128, 8, COLS], fp8, name=n) for n in wd
            }
            xh = xpool.tile([128, 8, S], fp8, name="xh")
            xl = xpool.tile([128, 8, S], fp8, name="xl")
            # Ordered + dual-queue loads: K weights and x seq-half 0 first so
            # the first projection can start ASAP; xl/wv on the Pool queue.
            def wload(n):
                eng = nc.sync if n in ("wkh", "wqh", "wvh") else nc.gpsimd
                eng.dma_start(out=wt[n], in_=wd[n].rearrange("(j p) c -> p j c", p=128))

            wload("wkh")
            wload("wkl")
            for half in range(2):
                s = slice(half * 1024, (half + 1) * 1024)
                for j in range(8):
                    nc.sync.dma_start(out=xh[:, j, s], in_=xh_d[j * 128 : (j + 1) * 128, s])
                    nc.gpsimd.dma_start(out=xl[:, j, s], in_=xl_d[j * 128 : (j + 1) * 128, s])
                if half == 0:
                    wload("wqh")
                    wload("wql")
            wload("wvh")
            wload("wvl")

            # q/k: per head-pair m: [128 (2 heads x 64 dh), S] bf16
            qtt = [qkt.tile([128, S], bf16, name=f"qt{m}") for m in range(NPAIR)]
            ktt = [qkt.tile([128, S], bf16, name=f"kt{m}") for m in range(NPAIR)]
            # v: per kb-pair: [128 seq, 2 kb, 8 heads, 65] bf16 (col 64 = 1)
            vt = [
                vpool.tile([128, 2, 8, 65], bf16, name=f"vt{i}")
                for i in range(NKB // 2)
            ]
            for i in range(NKB // 2):
                nc.vector.memset(vt[i][:, :, :, 64:65], 1.0)

            # ---- QKV projection (fp8 DR, 3 hi/lo terms) ----
            def dr_accum(ps, osl, wh_ap, wl_ap, xh_ap, xl_ap):
                """ps[:, osl] += full-D x@W via hi/lo fp8 DR passes."""
                n = 0
                for w_ap, x_ap in ((wh_ap, xh_ap), (wh_ap, xl_ap), (wl_ap, xh_ap)):
                    for j in range(4):
                        nc.tensor.matmul(
                            ps[:, osl],
                            lhsT=w_ap[:, 2 * j : 2 * j + 2, :],
                            rhs=x_ap[:, 2 * j : 2 * j + 2, :],
                            start=(n == 0),
                            stop=(n == 11),
                            perf_mode=DR,
                        )
                        n += 1

            def qk_proj_q(proj, m, quarter, dst, bias):
                wh, wl = wt[f"w{proj}h"], wt[f"w{proj}l"]
                c0 = m * 128
                ps = psum.tile([128, 512], f32, tag="cp", bufs=2, name="pj")
                for n in range(2):  # 256-seq chunks
                    s0 = quarter * 512 + n * 256
                    dr_accum(
                        ps,
                        slice(n * 256, (n + 1) * 256),
                        wh[:, :, c0 : c0 + 128],
                        wl[:, :, c0 : c0 + 128],
                        xh[:, :, s0 : s0 + 256],
                        xl[:, :, s0 : s0 + 256],
                    )
                nc.vector.tensor_scalar(
                    out=dst[:, quarter * 512 : (quarter + 1) * 512],
                    in0=ps,
                    scalar1=1.0 / WS,
                    scalar2=bias,
                    op0=mult,
                    op1=add,
                )

            def qk_proj(proj, m, dst, bias):
                for quarter in range(4):
                    qk_proj_q(proj, m, quarter, dst, bias)

            def v_proj(kb):
                wh, wl = wt["wvh"], wt["wvl"]
                ps = psum.tile([128, 512], f32, tag="cp", bufs=2, name="pj")
                s0 = kb * 128
                for n in range(2):  # 256-col chunks
                    dr_accum(
                        ps,
                        slice(n * 256, (n + 1) * 256),
                        xh[:, :, s0 : s0 + 128],
                        xl[:, :, s0 : s0 + 128],
                        wh[:, :, n * 256 : (n + 1) * 256],
                        wl[:, :, n * 256 : (n + 1) * 256],
                    )
                nc.vector.scalar_tensor_tensor(
                    out=vt[kb // 2][:, kb % 2, :, 0:64],
                    in0=ps.rearrange("p (h d) -> p h d", h=8),
                    scalar=1.0 / WS,
                    in1=bvb[:, 0, :].rearrange("p (h d) -> p h d", h=8),
                    op0=mult,
                    op1=add,
                )

            # ---- attention: phase A (scores+exp) / phase B (PV+normalize),
            # software-pipelined across units (A of unit u+1 outranks B of
            # unit u so the exp engines never starve).
            def unit_scores_exp(m, hh, qg, hooks=None):
                """hooks: {kb: callable} -- fill work (proj slabs) emitted
                inside the kb loop so the in-order PE stream never blocks
                long on the pj psum ring."""
                head = 2 * m + hh
                p0 = 64 * hh
                q0 = qg * QG
                kt_, qt_ = ktt[m], qtt[m]
                ees = {}

                def scores(kb):
                    sc = psum.tile([128, QG], f32, tag="sc", bufs=3, name="sc")
                    for qq in range(2):
                        nc.tensor.matmul(
                            sc[:, qq * 512 : (qq + 1) * 512],
                            lhsT=kt_[p0 : p0 + 64, kb * 128 : (kb + 1) * 128],
                            rhs=qt_[p0 : p0 + 64, q0 + qq * 512 : q0 + (qq + 1) * 512],
                            start=True,
                            stop=True,
                        )
                    return sc

                scs = [scores(0), scores(1), scores(2)]
                for kb in range(NKB):
                    kbp = kb // 2
                    if kbp not in ees:
                        ees[kbp] = epool.tile(
                            [128, 2, QG], bf16, tag="ee", name="ee"
                        )
                    ee = ees[kbp][:, kb % 2, :]
                    sc = scs[kb % 3]
                    if kb in ACT_KBS:
                        nc.scalar.activation(ee, sc, Exp, scale=0.125, bias=ln4[:, :])
                    else:
                        nc.vector._custom_dve(
                            exp16, out=ee, in0=sc, s0=EXP_A, s1=EXP_B, imm2=EXP_D
                        )
                    if kb < NKB - 3:
                        scs[kb % 3] = scores(kb + 3)
                    if hooks and kb in hooks:
                        for fn in hooks[kb]:  # inline: priority = emission pos
                            fn()
                return ees

            def b_pieces(m, hh, qg, ees):
                """PV + normalize of a finished unit as ~1us callables,
                hooked into the NEXT unit's kb loop. PV is per-512-q half,
                qt8-major: each ctx accumulator's 16-kb chain is one
                uninterrupted accumulation group (start=True clears
                has_written bits for the WHOLE psum bank)."""
                head = 2 * m + hh
                q0 = qg * QG
                state = {}

                def chain(half, q4):
                    def run():
                        if half not in state:
                            state[half] = psum.tile(
                                [128, 512], f32, tag="cp", bufs=2, name="ctx"
                            ).rearrange("p (q c) -> p q c", q=4)
                        ctx = state[half]
                        qt8 = half * 4 + q4
                        for kb in range(NKB):
                            nc.tensor.matmul(
                                ctx[:, q4, 0:65],
                                lhsT=ees[kb // 2][
                                    :, kb % 2, qt8 * 128 : (qt8 + 1) * 128
                                ],
                                rhs=vt[kb // 2][:, kb % 2, head, :],
                                start=(kb == 0),
                                stop=(kb == NKB - 1),
                            )
                    return run

                def finish(half):
                    def run():
                        ctx = state.pop(half)
                        rr = opool.tile([128, 4, 1], f32, tag="rr", name="rr")
                        nc.vector.reciprocal(rr, ctx[:, :, 64:65])
                        ob = opool.tile([128, 4, 64], f32, tag="ob", name="ob")
                        for q4 in range(4):
                            nc.vector.tensor_scalar_mul(
                                ob[:, q4, :], ctx[:, q4, 0:64], rr[:, q4, :]
                            )
                        nc.sync.dma_start(
                            out=out_d[
                                q0 + half * 512 : q0 + (half + 1) * 512,
                                head * 64 : (head + 1) * 64,
                            ].rearrange("(t p) c -> p t c", p=128),
                            in_=ob,
                        )
                    return run

                return [
                    chain(0, 0), chain(0, 1), chain(0, 2), chain(0, 3), finish(0),
                    chain(1, 0), chain(1, 1), chain(1, 2), chain(1, 3), finish(1),
                ]

            # ---- emission: everything inline; priority = emission order.
            # Prologue: just enough of K0/Q0 for unit 0 to start scoring.
            qk_proj_q("k", 0, 0, ktt[0], bk_t[:, 0:1])
            qk_proj_q("k", 0, 1, ktt[0], bk_t[:, 0:1])
            qk_proj_q("q", 0, 0, qtt[0], bq_t[:, 0:1])
            qk_proj_q("q", 0, 1, qtt[0], bq_t[:, 0:1])

            units = [
                (m, hh, qg)
                for m in range(NPAIR)
                for hh in range(2)
                for qg in range(NQG)
            ]

            def mk_v(kb):
                return lambda: v_proj(kb)

            def mk_qk(pr, mm, qq):
                dst = (ktt if pr == "k" else qtt)[mm]
                bias_t = bk_t if pr == "k" else bq_t
                return lambda: qk_proj_q(pr, mm, qq, dst, bias_t[:, mm : mm + 1])

            prev = None
            for idx, (m, hh, qg) in enumerate(units):
                hooks = {}

                def hook_add(kb, fn):
                    hooks.setdefault(kb, []).append(fn)

                if idx == 0:
                    for i, kb in enumerate((8, 9, 10, 11, 12, 13, 14, 15)):
                        hook_add(kb, mk_v(i))
                    hook_add(3, mk_qk("k", 0, 2))
                    hook_add(5, mk_qk("k", 0, 3))
                    hook_add(6, mk_qk("q", 0, 2))
                    hook_add(7, mk_qk("q", 0, 3))
                elif idx == 1:
                    for i, kb in enumerate((1, 2, 3, 4, 5, 6, 7, 8)):
                        hook_add(kb, mk_v(8 + i))
                elif idx % 4 == 2 and idx < 14:
                    mm = idx // 4 + 1
                    for qq in range(4):
                        hook_add(1 + 2 * qq, mk_qk("k", mm, qq))
                elif idx % 4 == 3 and idx < 15:
                    mm = idx // 4 + 1
                    for qq in range(4):
                        hook_add(1 + 2 * qq, mk_qk("q", mm, qq))

                if prev is not None:
                    pieces = b_pieces(*prev)
                    if idx == 1:
                        slots = [9, 9, 10, 10, 11, 12, 12, 13, 14, 15]
                    else:
                        slots = [6, 7, 8, 9, 10, 11, 12, 13, 14, 15]
                    for kb, fn in zip(slots, pieces):
                        hook_add(kb, fn)

                ees = unit_scores_exp(m, hh, qg, hooks=hooks)
                prev = (m, hh, qg, ees)

            for fn in b_pieces(*prev):
                fn()

    nc.compile()
    return nc


def _get_nc():
    if "nc" not in _CACHE:
        _CACHE["nc"] = _build()
    return _CACHE["nc"]


def _hl8(a):
    hi = a.astype(F8NP)
    lo = (a - hi.astype(np.float32)).astype(F8NP)
    return hi, lo


def _in_maps(x, Wq, bq, Wk, bk, Wv, bv):
    x = np.asarray(x, np.float32)
    maps = []
    for c in range(NCORES):
        b, hh = c // 2, c % 2
        cs = slice(hh * COLS, (hh + 1) * COLS)
        xT = np.ascontiguousarray(x[b].T)
        xhi, xlo = _hl8(xT)
        wqh, wql = _hl8(WS * np.asarray(Wq)[:, cs])
        wkh, wkl = _hl8(WS * np.asarray(Wk)[:, cs])
        wvh, wvl = _hl8(WS * np.asarray(Wv)[:, cs])
        maps.append(
            {
                "xh": xhi,
                "xl": xlo,
                "wqh": wqh,
                "wql": wql,
                "wkh": wkh,
                "wkl": wkl,
                "wvh": wvh,
                "wvl": wvl,
                "bq": np.ascontiguousarray(np.asarray(bq, np.float32)[cs]),
                "bk": np.ascontiguousarray(np.asarray(bk, np.float32)[cs]),
                "bv": np.ascontiguousarray(np.asarray(bv, np.float32)[cs]),
            }
        )
    return maps


def _run(inputs, trace=False):
    from concourse import bass_utils

    nc = _get_nc()
    res = bass_utils.run_bass_kernel_spmd(
        nc,
        _in_maps(**inputs),
        core_ids=list(range(NCORES)),
        trace=trace,
    )
    out = np.empty((B, S, D), np.float32)
    for c in range(NCORES):
        b, hh = c // 2, c % 2
        out[b, :, hh * COLS : (hh + 1) * COLS] = res.results[c]["out"]
    return out, res


def kernel(**inputs):
    out, _ = _run(inputs, trace=False)
    return out


if __name__ == "__main__":
    _get_nc()
    print("build ok")
